# revision 18
# baseline (speedup 1.0000x reference)
"""ExtractTensorPatches Trainium2 Bass kernel, v11: PE-dedup + SWDGE cast load.

Per-core layout (128 partitions): partition p = b2*64 + k, raw bf16 tile
[c][16 rows][512] = 24576 bf16 per partition.
  - Loads: one SWDGE (gpsimd, f32->bf16 casting) DMA per batch writes each
    input row exactly once into the partition's "top" slot (rows 8k..8k+7).
  - Bottom slots (rows 8k+8..8k+15 = partition p+1's top rows) come from
    the TensorEngine: a shift-matrix matmul (lhsT[k, p] = 1 iff k == p+1,
    so out[p] = rhs[p+1]) into bf16 PSUM, copied back by DVE at 2x mode.
    The row duplication consumes zero DMA-engine/HBM bandwidth, and the
    0/1 matmul is exact for bf16 values.
  - DVE gathers the im2col blocks (4x mode, pure bf16) into output-
    contiguous g tiles; stores are bf16 DMAs on the scalar queue only
    (dedicated queues: loads gpsimd, stores scalar -> no FIFO coupling).
HBM traffic per rep: 6.2MB f32 read + 12.2MB bf16 write = 18.4MB.
Host upcasts the bf16 result to f32 (harness tolerance 2e-2 is ~5x the
worst-case bf16 rounding error of 2^-9).
"""

import sys

import numpy as np

if "/opt/trn_rl_repo" not in sys.path:
    sys.path.insert(0, "/opt/trn_rl_repo")

B, C, H, W = 16, 3, 512, 512
WH, WW, SH, SW = 16, 16, 8, 8
HO = (H - WH) // SH + 1  # 63
WO = (W - WW) // SW + 1  # 63
N = HO * WO  # 3969
NCORES = 8
BPC = B // NCORES  # 2
IMG = C * H * W
PATCH = C * WH * WW  # 768
TOP_F = SH * W  # 4096 elements per (partition, channel) row-octet
RAW_F = C * 2 * TOP_F  # 24576 bf16 elements per partition ([c][16][512])
NPART = 128
BLOCKS = [(0, 32), (32, 31)]
MM = 512  # matmul moving free dim
PS_F = 2048  # psum tile free dim (4 matmuls per tile)

_CACHE = {}
LAST_RESULTS = None


def _shift_lhsT() -> np.ndarray:
    """lhsT[k, p] = 1 iff k == p+1  (so out[p] = rhs[p+1])."""
    m = np.zeros((128, 128), dtype=np.float32)
    for p in range(127):
        m[p + 1, p] = 1.0
    import ml_dtypes

    return m.astype(ml_dtypes.bfloat16)


def _build(reps: int = 1):
    import concourse.bass as bass
    import concourse.bacc as bacc
    import concourse.mybir as mybir
    from concourse.tile import TileContext

    nc = bacc.Bacc("TRN2", target_bir_lowering=False, debug=False)
    x = nc.dram_tensor("x", [BPC, C, H, W], mybir.dt.float32, kind="ExternalInput").ap()
    sh = nc.dram_tensor(
        "shiftT", [128, 128], mybir.dt.bfloat16, kind="ExternalInput"
    ).ap()
    y = nc.dram_tensor(
        "y", [BPC, N, C, WH, WW], mybir.dt.bfloat16, kind="ExternalOutput"
    ).ap()

    with TileContext(nc) as tc:
        with (
            tc.tile_pool(name="raw", bufs=1) as rawp,
            tc.tile_pool(name="g", bufs=3) as gp,
            tc.tile_pool(name="w", bufs=1) as wp,
            tc.tile_pool(name="ps", bufs=2, space="PSUM") as psp,
        ):
            shiftT = wp.tile([128, 128], mybir.dt.bfloat16, name="shiftT", tag="w")
            nc.sync.dma_start(out=shiftT[:, :], in_=sh)

            for _rep in range(reps):
                raw = rawp.tile([NPART, RAW_F], mybir.dt.bfloat16, name="raw", tag="raw")

                # Loads (gpsimd/SWDGE queue, f32 -> bf16 cast during DMA):
                # each input row exactly once, into the top slots.
                for b2 in range(BPC):
                    src = bass.AP(
                        tensor=x.tensor,
                        offset=b2 * IMG,
                        ap=[[TOP_F, 64], [H * W, C], [1, TOP_F]],
                    )
                    dst = bass.AP(
                        tensor=raw.tensor,
                        offset=b2 * 64 * RAW_F,
                        ap=[[RAW_F, 64], [2 * TOP_F, C], [1, TOP_F]],
                    )
                    nc.gpsimd.dma_start(out=dst, in_=src)

                # Bottom slots: shift by one partition via PE (bf16 PSUM),
                # then DVE copy back.
                for c in range(C):
                    for h2 in range(TOP_F // PS_F):
                        ps = psp.tile([128, PS_F], mybir.dt.float32, tag="ps")
                        for k in range(PS_F // MM):
                            rhs = bass.AP(
                                tensor=raw.tensor,
                                offset=c * 2 * TOP_F + h2 * PS_F + k * MM,
                                ap=[[RAW_F, NPART], [1, MM]],
                            )
                            nc.tensor.matmul(
                                ps[:, k * MM : (k + 1) * MM],
                                shiftT[:, :],
                                rhs,
                                start=True,
                                stop=True,
                            )
                        bot = bass.AP(
                            tensor=raw.tensor,
                            offset=c * 2 * TOP_F + TOP_F + h2 * PS_F,
                            ap=[[RAW_F, NPART], [1, PS_F]],
                        )
                        nc.vector.tensor_copy(out=bot, in_=ps[:, :])

                # Gather + store.
                for (w0, wb) in BLOCKS:
                    g = gp.tile([NPART, wb * PATCH], mybir.dt.bfloat16, tag="g")
                    for c in range(C):
                        in_ap = bass.AP(
                            tensor=raw.tensor,
                            offset=c * 2 * TOP_F + SW * w0,
                            ap=[[RAW_F, NPART - 1], [SW, wb], [W, WH], [1, WW]],
                        )
                        out_ap = bass.AP(
                            tensor=g.tensor,
                            offset=c * WH * WW,
                            ap=[[wb * PATCH, NPART - 1], [PATCH, wb], [WW, WH], [1, WW]],
                        )
                        nc.vector.tensor_copy(out=out_ap, in_=in_ap)
                    for b2 in range(BPC):
                        dst = bass.AP(
                            tensor=y.tensor,
                            offset=b2 * N * PATCH + w0 * PATCH,
                            ap=[[WO * PATCH, HO], [1, wb * PATCH]],
                        )
                        nc.scalar.dma_start(
                            out=dst, in_=g[b2 * 64 : b2 * 64 + HO, :]
                        )
    nc.compile()
    return nc


def _get_nc():
    if "nc" not in _CACHE:
        _CACHE["nc"] = _build()
    return _CACHE["nc"]


def _extra_inputs() -> dict:
    return {"shiftT": _shift_lhsT()}


EXTRA_INPUTS = _extra_inputs


def kernel(x: np.ndarray) -> np.ndarray:
    global LAST_RESULTS
    from concourse import bass_utils

    x = np.ascontiguousarray(np.asarray(x), dtype=np.float32)
    assert x.shape == (B, C, H, W), x.shape

    nc = _get_nc()
    shift = _shift_lhsT()
    in_maps = [
        {"x": np.ascontiguousarray(x[k * BPC : (k + 1) * BPC]), "shiftT": shift}
        for k in range(NCORES)
    ]
    res = bass_utils.run_bass_kernel_spmd(nc, in_maps, core_ids=list(range(NCORES)))
    LAST_RESULTS = res
    out = np.concatenate(
        [np.asarray(res.results[k]["y"]).astype(np.float32) for k in range(NCORES)],
        axis=0,
    )
    return out.reshape(B, N, C, WH, WW)


# revision 20
# speedup vs baseline: 1.0874x; 1.0874x over previous
"""ExtractTensorPatches Trainium2 Bass kernel, v8: PE-dedup.

Per-core layout (128 partitions): partition p = b2*64 + k.
  - "staging" f32 tile: loaded from HBM with each input row exactly once:
    partition p holds rows 8k..8k+7 of batch b2, all 3 channels
    ([c][8 rows][512] = 12288 f32).
  - "raw" bf16 tile ([c][16 rows][512] = 24576 bf16): top slot (rows
    8k..8k+7) is a GpSimd cast-copy of staging (keeps DVE free for the PSUM copies and gathers); bottom slot (rows 8k+8..
    8k+15 = partition p+1's top rows) is produced by the TensorEngine:
    a shift-matrix matmul (lhsT[k, p] = 1 iff k == p+1, so out[p] =
    rhs[p+1]) through PSUM, copied+cast to bf16 by DVE. The row
    duplication therefore consumes zero DMA-engine/HBM bandwidth.
  - DVE gathers the im2col blocks (4x mode, pure bf16), stores are
    output-contiguous bf16 DMAs.
HBM/DMA traffic per rep: 6.2MB f32 load (sync queue) + 12.2MB bf16 store
(scalar queue) = 18.4MB, vs 24.6MB for the duplicated-load variant.
Host upcasts the bf16 result to f32.
"""

import sys

import numpy as np

if "/opt/trn_rl_repo" not in sys.path:
    sys.path.insert(0, "/opt/trn_rl_repo")

B, C, H, W = 16, 3, 512, 512
WH, WW, SH, SW = 16, 16, 8, 8
HO = (H - WH) // SH + 1  # 63
WO = (W - WW) // SW + 1  # 63
N = HO * WO  # 3969
NCORES = 8
BPC = B // NCORES  # 2
IMG = C * H * W
PATCH = C * WH * WW  # 768
TOP_F = SH * W  # 4096 elements per (partition, channel) row-octet
STG_F = C * TOP_F  # 12288 f32 staging elements per partition
RAW_F = C * 2 * TOP_F  # 24576 bf16 elements per partition ([c][16][512])
NPART = 128
BLOCKS = [(0, 32), (32, 31)]
MM = 512  # matmul moving free dim
PS_F = 2048  # psum tile free dim (4 matmuls per tile)

_CACHE = {}
LAST_RESULTS = None


def _shift_lhsT() -> np.ndarray:
    """lhsT[k, p] = 1 iff k == p+1  (so out[p] = rhs[p+1])."""
    m = np.zeros((128, 128), dtype=np.float32)
    for p in range(127):
        m[p + 1, p] = 1.0
    import ml_dtypes

    return m.astype(ml_dtypes.bfloat16)


def _build(reps: int = 1):
    import concourse.bass as bass
    import concourse.bacc as bacc
    import concourse.mybir as mybir
    from concourse.tile import TileContext

    nc = bacc.Bacc("TRN2", target_bir_lowering=False, debug=False)
    x = nc.dram_tensor("x", [BPC, C, H, W], mybir.dt.float32, kind="ExternalInput").ap()
    sh = nc.dram_tensor(
        "shiftT", [128, 128], mybir.dt.bfloat16, kind="ExternalInput"
    ).ap()
    y = nc.dram_tensor(
        "y", [BPC, N, C, WH, WW], mybir.dt.bfloat16, kind="ExternalOutput"
    ).ap()

    with TileContext(nc) as tc:
        with (
            tc.tile_pool(name="stg", bufs=1) as stgp,
            tc.tile_pool(name="raw", bufs=1) as rawp,
            tc.tile_pool(name="g", bufs=2) as gp,
            tc.tile_pool(name="w", bufs=1) as wp,
            tc.tile_pool(name="ps", bufs=2, space="PSUM") as psp,
        ):
            shiftT = wp.tile([128, 128], mybir.dt.bfloat16, name="shiftT", tag="w")
            nc.sync.dma_start(out=shiftT[:, :], in_=sh)

            for _rep in range(reps):
                stg = stgp.tile([NPART, STG_F], mybir.dt.float32, name="stg", tag="stg")
                raw = rawp.tile([NPART, RAW_F], mybir.dt.bfloat16, name="raw", tag="raw")

                # Loads (sync queue): each input row exactly once.
                for b2 in range(BPC):
                    src = bass.AP(
                        tensor=x.tensor,
                        offset=b2 * IMG,
                        ap=[[TOP_F, 64], [H * W, C], [1, TOP_F]],
                    )
                    nc.sync.dma_start(out=stg[b2 * 64 : (b2 + 1) * 64, :], in_=src)

                # Top slots: cast staging f32 -> raw bf16.
                cast_in = bass.AP(
                    tensor=stg.tensor,
                    offset=0,
                    ap=[[STG_F, NPART], [TOP_F, C], [1, TOP_F]],
                )
                cast_out = bass.AP(
                    tensor=raw.tensor,
                    offset=0,
                    ap=[[RAW_F, NPART], [2 * TOP_F, C], [1, TOP_F]],
                )
                nc.gpsimd.tensor_copy(out=cast_out, in_=cast_in)

                # Bottom slots: shift by one partition via PE, then cast.
                for c in range(C):
                    for h2 in range(TOP_F // PS_F):
                        ps = psp.tile([128, PS_F], mybir.dt.float32, tag="ps")
                        for k in range(PS_F // MM):
                            rhs = bass.AP(
                                tensor=raw.tensor,
                                offset=c * 2 * TOP_F + h2 * PS_F + k * MM,
                                ap=[[RAW_F, NPART], [1, MM]],
                            )
                            nc.tensor.matmul(
                                ps[:, k * MM : (k + 1) * MM],
                                shiftT[:, :],
                                rhs,
                                start=True,
                                stop=True,
                            )
                        bot = bass.AP(
                            tensor=raw.tensor,
                            offset=c * 2 * TOP_F + TOP_F + h2 * PS_F,
                            ap=[[RAW_F, NPART], [1, PS_F]],
                        )
                        nc.vector.tensor_copy(out=bot, in_=ps[:, :])

                # Gather + store.
                for (w0, wb) in BLOCKS:
                    g = gp.tile([NPART, wb * PATCH], mybir.dt.bfloat16, tag="g")
                    for c in range(C):
                        in_ap = bass.AP(
                            tensor=raw.tensor,
                            offset=c * 2 * TOP_F + SW * w0,
                            ap=[[RAW_F, NPART - 1], [SW, wb], [W, WH], [1, WW]],
                        )
                        out_ap = bass.AP(
                            tensor=g.tensor,
                            offset=c * WH * WW,
                            ap=[[wb * PATCH, NPART - 1], [PATCH, wb], [WW, WH], [1, WW]],
                        )
                        nc.vector.tensor_copy(out=out_ap, in_=in_ap)
                    for b2 in range(BPC):
                        dst = bass.AP(
                            tensor=y.tensor,
                            offset=b2 * N * PATCH + w0 * PATCH,
                            ap=[[WO * PATCH, HO], [1, wb * PATCH]],
                        )
                        nc.scalar.dma_start(
                            out=dst, in_=g[b2 * 64 : b2 * 64 + HO, :]
                        )
    nc.compile()
    return nc


def _get_nc():
    if "nc" not in _CACHE:
        _CACHE["nc"] = _build()
    return _CACHE["nc"]


def _extra_inputs() -> dict:
    return {"shiftT": _shift_lhsT()}


EXTRA_INPUTS = _extra_inputs


def kernel(x: np.ndarray) -> np.ndarray:
    global LAST_RESULTS
    from concourse import bass_utils

    x = np.ascontiguousarray(np.asarray(x), dtype=np.float32)
    assert x.shape == (B, C, H, W), x.shape

    nc = _get_nc()
    shift = _shift_lhsT()
    in_maps = [
        {"x": np.ascontiguousarray(x[k * BPC : (k + 1) * BPC]), "shiftT": shift}
        for k in range(NCORES)
    ]
    res = bass_utils.run_bass_kernel_spmd(nc, in_maps, core_ids=list(range(NCORES)))
    LAST_RESULTS = res
    out = np.concatenate(
        [np.asarray(res.results[k]["y"]).astype(np.float32) for k in range(NCORES)],
        axis=0,
    )
    return out.reshape(B, N, C, WH, WW)
